# revision 9
# baseline (speedup 1.0000x reference)
"""Trainium2 Bass kernel for EnhancedFastKANLayer.

Reference computation (B=16384, D=O=512, G=8 grids):
    x_norm = (x - mean) * rsqrt(var + eps) * gamma + beta          # BN inference
    basis[b,d,g] = exp(-((x_norm[b,d] - grid[g]) / denom)^2)       # RBF expansion
    out = basis.reshape(B, D*G) @ W_spline + b_spline
        + relu(x) @ W_base + b_base + x

Strategy:
  - Data parallel: batch 16384 sharded 8 ways (2048 rows/core); weights
    replicated. No collectives.
  - All on-chip compute happens in the transposed layout [feature, batch]:
    the output is produced as out_T [O, B_shard] and transposed back on the
    host. This makes BN/basis per-partition-scalar ops, lets the spline
    matmul consume basis tiles directly as the moving operand, and makes the
    residual (+x) a single extra identity matmul into the same PSUM
    accumulator.
  - RBF via ScalarE Derivative_Erf: d/dx erf(x) = 2/sqrt(pi)*exp(-x^2), so
    basis_g = sqrt(pi)/2 * Derivative_Erf(u - c_g) -- ONE ACT op per grid
    (bias supplies -c_g), with the sqrt(pi)/2 constant folded into W_spline
    on the host.  HW-probed: rel err 1.3e-5, saturates cleanly to 0 for
    |x| > 6, no NaN/Inf out to |x|=24.
  - x is pre-cast to fp16 AND pre-transposed to [D, B_shard] on the host:
    fp16 keeps the BN input error at 2^-11 relative, and host-side
    transposition means every device DMA is a plain contiguous copy (no
    xbar DMA-transpose -> no transpose<->copy mode serialization on the
    DMA rings).
  - W_spline is pre-reordered on the host to K-order (dt, g, d_in) matching
    the order basis tiles are produced on chip, cast to bf16.
  - Matmul: out_T[o_sub, b] accumulates 36 matmuls per PSUM tile:
    32 spline K-chunks + 4 relu(x)@W_base K-chunks; the +x residual and
    output bias ride the DVE epilogue that evacuates PSUM.
    All 8 PSUM banks accumulate one chunk concurrently, K-blocked (blocks of
    6 K-chunks swept across all banks) so the PE starts as soon as the first
    6 basis tiles exist instead of waiting for the whole chunk's basis.
"""

import numpy as np
import ml_dtypes
from contextlib import ExitStack

import concourse.bass as bass
import concourse.tile as tile
from concourse import bacc, mybir
from concourse._compat import with_exitstack
from concourse.bass_utils import run_bass_kernel_spmd
from concourse.masks import make_identity

N_CORES = 8
BATCH, IN_DIM, OUT_DIM, G = 16384, 512, 512, 8
B_SHARD = BATCH // N_CORES          # 2048
B_CHUNK = 1024                      # batch columns processed per chunk
GRID_MIN, GRID_MAX, BN_EPS = -2.0, 2.0, 1e-3
DENOM = (GRID_MAX - GRID_MIN) / G   # 0.5
N_DT = IN_DIM // 128                # 4 d-tiles
K_SPLINE = N_DT * G                 # 32 spline K-chunks
K_BASE = N_DT                       # 4 base K-chunks
N_OSUB = OUT_DIM // 128             # 4 output partition tiles

F32 = mybir.dt.float32
F16 = mybir.dt.float16
BF16 = mybir.dt.bfloat16


def _grid_consts():
    grid = np.linspace(GRID_MIN, GRID_MAX, G, dtype=np.float32)
    c = (grid / np.float32(DENOM)).astype(np.float32)        # grid in u-units
    return c


def _col(vec_ap, start, p=128):
    """View rows [start, start+p) of a 1-D DRAM tensor as a [p, 1] AP."""
    return bass.AP(
        tensor=vec_ap.tensor,
        offset=vec_ap.offset + start,
        ap=[[1, p], [0, 1]],
    )


@with_exitstack
def _body(ctx, tc, x16t, w_sp, w_b, params, out_t, b_shard, b_chunk):
    nc = tc.nc
    n_chunks = b_shard // b_chunk
    n_bh = b_chunk // 512            # 512-wide moving-operand slices
    k_total = K_SPLINE + K_BASE      # residual rides the DVE epilogue
    KB = 6                           # K-chunks per PE block
    K_LAST = 30                      # kc >= K_LAST run per-bank, fused w/ epilogue
    W_SLICE = 8                      # spline weight K-chunks per DMA

    const_pool = ctx.enter_context(tc.tile_pool(name="const", bufs=1))
    w_pool = ctx.enter_context(tc.tile_pool(name="w", bufs=1))
    xt_pool = ctx.enter_context(tc.tile_pool(name="xt", bufs=2 * N_DT))
    u_pool = ctx.enter_context(tc.tile_pool(name="u", bufs=3))
    # spline basis tiles: 32 per chunk stay resident through the chunk's
    # matmul phase; extra slots let the next chunk's production run ahead.
    basis_pool = ctx.enter_context(tc.tile_pool(name="basis", bufs=K_SPLINE + 12))
    relu_pool = ctx.enter_context(tc.tile_pool(name="relu", bufs=2 * N_DT))
    psum_pool = ctx.enter_context(
        tc.tile_pool(name="psum", bufs=8, space="PSUM"))
    out_pool = ctx.enter_context(tc.tile_pool(name="outs", bufs=4))

    # ---- identity first: it feeds the PE warm-up matmuls ----
    ident = const_pool.tile([128, 128], F16)
    make_identity(nc, ident)
    # preload the ACT function table (walrus inserts the ACT_TABLE_LOAD
    # before this first ACTIVATE) so it overlaps the input DMAs
    scratch = const_pool.tile([128, 1], F16)
    nc.vector.memset(scratch, 0.0)
    nc.scalar.activation(out=scratch, in_=scratch,
                         func=mybir.ActivationFunctionType.Derivative_Erf)

    # ---- weights + params.  The first PE block's weights (kc 0..5) go on
    # the fast sync (HWDGE) queue ahead of the xT loads; the rest stream on
    # the gpsimd (SWDGE) queue and stay ahead of PE block consumption ----
    w_tile = w_pool.tile([128, K_SPLINE, OUT_DIM], F16)
    nc.sync.dma_start(out=w_tile[:, 0:KB, :], in_=w_sp[:, 0:KB, :])
    params_sb = const_pool.tile([128, N_DT + N_DT + N_OSUB + G], F32)
    nc.gpsimd.dma_start(out=params_sb, in_=params)
    uscale_sb = params_sb[:, 0:N_DT]
    ushift_sb = params_sb[:, N_DT:2 * N_DT]
    bias_sb = params_sb[:, 2 * N_DT:2 * N_DT + N_OSUB]
    negc = params_sb[:, 2 * N_DT + N_OSUB:]
    for ws in range(KB, K_SPLINE, W_SLICE):
        we = min(ws + W_SLICE, K_SPLINE)
        nc.gpsimd.dma_start(out=w_tile[:, ws:we, :], in_=w_sp[:, ws:we, :])
    wb_tile = w_pool.tile([128, K_BASE, OUT_DIM], F16)
    nc.gpsimd.dma_start(out=wb_tile, in_=w_b)

    def emit_producers(ch):
        b0 = ch * b_chunk
        xts, relus, basis = [], [], []
        for dt in range(N_DT):
            xt = xt_pool.tile([128, b_chunk], F16, tag="xt")
            nc.sync.dma_start(
                out=xt,
                in_=x16t[dt * 128:(dt + 1) * 128, b0:b0 + b_chunk],
            )
            xts.append(xt)
            u = u_pool.tile([128, b_chunk], F32, tag="u")
            nc.vector.tensor_scalar(
                out=u, in0=xt,
                scalar1=uscale_sb[:, dt:dt + 1], scalar2=ushift_sb[:, dt:dt + 1],
                op0=mybir.AluOpType.mult, op1=mybir.AluOpType.add,
            )
            rl = relu_pool.tile([128, b_chunk], F16, tag="relu")
            nc.vector.tensor_scalar_max(out=rl, in0=xt, scalar1=0.0)
            relus.append(rl)
            for g in range(G):
                bt = basis_pool.tile([128, b_chunk], F16, tag="basis")
                # basis_g = sqrt(pi)/2 * d/dx erf(u - c_g); constant folded
                # into W_spline host-side.
                nc.scalar.activation(
                    out=bt, in_=u,
                    func=mybir.ActivationFunctionType.Derivative_Erf,
                    bias=negc[:, g:g + 1],
                )
                basis.append(bt)
        return xts, relus, basis

    def operands(kc, osub, xts, relus, basis):
        if kc < K_SPLINE:
            return w_tile[:, kc, osub * 128:(osub + 1) * 128], basis[kc]
        dt = kc - K_SPLINE
        return wb_tile[:, dt, osub * 128:(osub + 1) * 128], relus[dt]

    def emit_main_blocks(ch, psums, xts, relus, basis):
        for kb in range(0, K_LAST, KB):
            for osub in range(N_OSUB):
                for bh in range(n_bh):
                    for kc in range(kb, kb + KB):
                        lhsT, rhs = operands(kc, osub, xts, relus, basis)
                        nc.tensor.matmul(
                            psums[osub * n_bh + bh], lhsT=lhsT,
                            rhs=rhs[:, bh * 512:(bh + 1) * 512],
                            start=(kc == 0), stop=False)

    def emit_final_block(ch, psums, xts, relus, basis):
        b0 = ch * b_chunk
        for osub in range(N_OSUB):
            for bh in range(n_bh):
                ps = psums[osub * n_bh + bh]
                for kc in range(K_LAST, k_total):
                    lhsT, rhs = operands(kc, osub, xts, relus, basis)
                    nc.tensor.matmul(
                        ps, lhsT=lhsT, rhs=rhs[:, bh * 512:(bh + 1) * 512],
                        start=False, stop=(kc == k_total - 1))
                ot = out_pool.tile([128, 512], F32, tag="ot")
                # residual (+x) and output bias
                nc.vector.tensor_add(
                    out=ot, in0=ps,
                    in1=xts[osub][:, bh * 512:(bh + 1) * 512])
                nc.vector.tensor_scalar_add(
                    out=ot, in0=ot, scalar1=bias_sb[:, osub:osub + 1])
                nc.sync.dma_start(
                    out=out_t[osub * 128:(osub + 1) * 128,
                              b0 + bh * 512:b0 + (bh + 1) * 512],
                    in_=ot)

    # Emission order keeps each engine's in-order stream free of
    # cross-chunk serialization: chunk ch+1's DVE/ACT producer ops are
    # emitted BEFORE chunk ch's final block + epilogue (which wait on ch's
    # last matmuls).
    def alloc_psums(ch):
        return [psum_pool.tile([128, 512], F32, tag="ps", name=f"ps{ch}_{i}")
                for i in range(N_OSUB * n_bh)]

    psums0 = alloc_psums(0)
    # PE warm-up: ~32 dependency-free matmuls into psums0[0] release the HAM
    # clock throttle (~3.4us of sustained PE activity) before the first real
    # matmul arrives; the real kc==0 matmul has start=True, which resets the
    # bank, so the junk results never reach the output.
    for _ in range(32):
        nc.tensor.matmul(psums0[0][:, 0:128], lhsT=ident, rhs=ident,
                         start=True, stop=False)

    prod = emit_producers(0)
    psums = psums0
    for ch in range(n_chunks):
        emit_main_blocks(ch, psums, *prod)
        cur_prod, cur_psums = prod, psums
        if ch + 1 < n_chunks:
            prod = emit_producers(ch + 1)
            psums = alloc_psums(ch + 1)
        emit_final_block(ch, cur_psums, *cur_prod)


def build_program(b_shard=B_SHARD, b_chunk=B_CHUNK):
    nc = bacc.Bacc("TRN2", target_bir_lowering=False, debug=False,
                   num_devices=N_CORES)
    x16t = nc.dram_tensor("x16t", [IN_DIM, b_shard], F16,
                          kind="ExternalInput").ap()
    w_sp = nc.dram_tensor("w_sp", [128, K_SPLINE, OUT_DIM], F16,
                          kind="ExternalInput").ap()
    w_b = nc.dram_tensor("w_base", [128, K_BASE, OUT_DIM], F16,
                         kind="ExternalInput").ap()
    n_par = 2 * N_DT + N_OSUB + G
    params = nc.dram_tensor("params", [128, n_par], F32,
                            kind="ExternalInput").ap()
    out_t = nc.dram_tensor("out_t", [OUT_DIM, b_shard], F32,
                           kind="ExternalOutput").ap()
    with tile.TileContext(nc) as tc:
        _body(tc, x16t, w_sp, w_b, params, out_t, b_shard, b_chunk)
    nc.compile()
    return nc


def make_in_maps(x, gamma, beta, moving_mean, moving_var, W_spline, b_spline,
                 W_base, b_base, n_cores=N_CORES):
    """Host-side preprocessing + per-core input shards."""
    x = np.asarray(x, dtype=np.float32)
    gamma = np.asarray(gamma, dtype=np.float32)
    beta = np.asarray(beta, dtype=np.float32)
    moving_mean = np.asarray(moving_mean, dtype=np.float32)
    moving_var = np.asarray(moving_var, dtype=np.float32)
    W_spline = np.asarray(W_spline, dtype=np.float32)
    W_base = np.asarray(W_base, dtype=np.float32)
    b_spline = np.asarray(b_spline, dtype=np.float32)
    b_base = np.asarray(b_base, dtype=np.float32)

    scale = gamma / np.sqrt(moving_var + np.float32(BN_EPS))
    shift = beta - moving_mean * scale
    uscale = (scale / np.float32(DENOM)).astype(np.float32)
    ushift = (shift / np.float32(DENOM)).astype(np.float32)

    x16t = np.ascontiguousarray(x.T.astype(np.float16))  # [D, B]
    # K-order on chip is (dt, g, d_in): kc = dt*8+g covers d in
    # [dt*128, (dt+1)*128) at grid g.  W_spline rows are (d, g)-ordered.
    w_r = (W_spline.reshape(N_DT, 128, G, OUT_DIM)
           .transpose(0, 2, 1, 3)            # (dt, g, d_in, o)
           .reshape(K_SPLINE, 128, OUT_DIM)
           .transpose(1, 0, 2))              # (d_in, kc, o)
    w_sp = np.ascontiguousarray(w_r * np.float32(np.sqrt(np.pi) / 2.0)
                               ).astype(np.float16)
    w_b = np.ascontiguousarray(
        W_base.reshape(K_BASE, 128, OUT_DIM).transpose(1, 0, 2)
    ).astype(np.float16)
    bias_o = (b_spline + b_base).astype(np.float32)
    c = _grid_consts()
    params = np.empty((128, 2 * N_DT + N_OSUB + G), np.float32)
    params[:, 0:N_DT] = uscale.reshape(N_DT, 128).T
    params[:, N_DT:2 * N_DT] = ushift.reshape(N_DT, 128).T
    params[:, 2 * N_DT:2 * N_DT + N_OSUB] = bias_o.reshape(N_OSUB, 128).T
    params[:, 2 * N_DT + N_OSUB:] = -c[None, :]

    b_shard = x.shape[0] // n_cores
    return [
        {
            "x16t": np.ascontiguousarray(
                x16t[:, ci * b_shard:(ci + 1) * b_shard]),
            "w_sp": w_sp,
            "w_base": w_b,
            "params": params,
        }
        for ci in range(n_cores)
    ]


_PROGRAM = None


def kernel(x, gamma, beta, moving_mean, moving_var, W_spline, b_spline,
           W_base, b_base):
    global _PROGRAM
    if _PROGRAM is None:
        _PROGRAM = build_program()
    in_maps = make_in_maps(x, gamma, beta, moving_mean, moving_var,
                           W_spline, b_spline, W_base, b_base)
    res = run_bass_kernel_spmd(_PROGRAM, in_maps, core_ids=list(range(N_CORES)))
    out = np.concatenate(
        [np.ascontiguousarray(res.results[ci]["out_t"].T)
         for ci in range(N_CORES)], axis=0)
    return out.astype(np.float32)


# revision 10
# speedup vs baseline: 1.1513x; 1.1513x over previous
"""Trainium2 Bass kernel for EnhancedFastKANLayer.

Reference computation (B=16384, D=O=512, G=8 grids):
    x_norm = (x - mean) * rsqrt(var + eps) * gamma + beta          # BN inference
    basis[b,d,g] = exp(-((x_norm[b,d] - grid[g]) / denom)^2)       # RBF expansion
    out = basis.reshape(B, D*G) @ W_spline + b_spline
        + relu(x) @ W_base + b_base + x

Strategy:
  - Data parallel: batch 16384 sharded 8 ways (2048 rows/core); weights
    replicated. No collectives.
  - All on-chip compute happens in the transposed layout [feature, batch]:
    the output is produced as out_T [O, B_shard] and transposed back on the
    host. This makes BN/basis per-partition-scalar ops, lets the spline
    matmul consume basis tiles directly as the moving operand, and makes the
    residual (+x) a single extra identity matmul into the same PSUM
    accumulator.
  - RBF via ScalarE Derivative_Erf: d/dx erf(x) = 2/sqrt(pi)*exp(-x^2), so
    basis_g = sqrt(pi)/2 * Derivative_Erf(u - c_g) -- ONE ACT op per grid
    (bias supplies -c_g), with the sqrt(pi)/2 constant folded into W_spline
    on the host.  HW-probed: rel err 1.3e-5, saturates cleanly to 0 for
    |x| > 6, no NaN/Inf out to |x|=24.
  - x is pre-cast to fp16 AND pre-transposed to [D, B_shard] on the host:
    fp16 keeps the BN input error at 2^-11 relative, and host-side
    transposition means every device DMA is a plain contiguous copy (no
    xbar DMA-transpose -> no transpose<->copy mode serialization on the
    DMA rings).
  - W_spline is pre-reordered on the host to K-order (dt, g, d_in) matching
    the order basis tiles are produced on chip, cast to bf16.
  - Matmul: out_T[o_sub, b] accumulates 37 matmuls per PSUM tile:
    32 spline K-chunks + 4 relu(x)@W_base K-chunks + 1 identity*xT residual.
    All 8 PSUM banks accumulate one chunk concurrently, K-blocked (blocks of
    6 K-chunks swept across all banks) so the PE starts as soon as the first
    6 basis tiles exist instead of waiting for the whole chunk's basis.
"""

import numpy as np
import ml_dtypes
from contextlib import ExitStack

import concourse.bass as bass
import concourse.tile as tile
from concourse import bacc, mybir
from concourse._compat import with_exitstack
from concourse.bass_utils import run_bass_kernel_spmd
from concourse.masks import make_identity

N_CORES = 8
BATCH, IN_DIM, OUT_DIM, G = 16384, 512, 512, 8
B_SHARD = BATCH // N_CORES          # 2048
B_CHUNK = 1024                      # batch columns processed per chunk
GRID_MIN, GRID_MAX, BN_EPS = -2.0, 2.0, 1e-3
DENOM = (GRID_MAX - GRID_MIN) / G   # 0.5
N_DT = IN_DIM // 128                # 4 d-tiles
K_SPLINE = N_DT * G                 # 32 spline K-chunks
K_BASE = N_DT                       # 4 base K-chunks
N_OSUB = OUT_DIM // 128             # 4 output partition tiles

F32 = mybir.dt.float32
F16 = mybir.dt.float16
BF16 = mybir.dt.bfloat16


def _grid_consts():
    grid = np.linspace(GRID_MIN, GRID_MAX, G, dtype=np.float32)
    c = (grid / np.float32(DENOM)).astype(np.float32)        # grid in u-units
    return c


def _col(vec_ap, start, p=128):
    """View rows [start, start+p) of a 1-D DRAM tensor as a [p, 1] AP."""
    return bass.AP(
        tensor=vec_ap.tensor,
        offset=vec_ap.offset + start,
        ap=[[1, p], [0, 1]],
    )


@with_exitstack
def _body(ctx, tc, x16t, w_sp, w_b, params, out_t, b_shard, b_chunk):
    nc = tc.nc
    n_chunks = b_shard // b_chunk
    n_bh = b_chunk // 512            # 512-wide moving-operand slices
    k_total = K_SPLINE + K_BASE + 1  # + residual identity matmul
    KB = 6                           # K-chunks per PE block
    K_LAST = 30                      # kc >= K_LAST run per-bank, fused w/ epilogue
    W_SLICE = 8                      # spline weight K-chunks per DMA

    const_pool = ctx.enter_context(tc.tile_pool(name="const", bufs=1))
    w_pool = ctx.enter_context(tc.tile_pool(name="w", bufs=1))
    xt_pool = ctx.enter_context(tc.tile_pool(name="xt", bufs=2 * N_DT))
    u_pool = ctx.enter_context(tc.tile_pool(name="u", bufs=3))
    # spline basis tiles: 32 per chunk stay resident through the chunk's
    # matmul phase; extra slots let the next chunk's production run ahead.
    basis_pool = ctx.enter_context(tc.tile_pool(name="basis", bufs=K_SPLINE + 12))
    relu_pool = ctx.enter_context(tc.tile_pool(name="relu", bufs=2 * N_DT))
    psum_pool = ctx.enter_context(
        tc.tile_pool(name="psum", bufs=8, space="PSUM"))
    out_pool = ctx.enter_context(tc.tile_pool(name="outs", bufs=4))

    # ---- identity first: it feeds the PE warm-up matmuls ----
    ident = const_pool.tile([128, 128], F16)
    make_identity(nc, ident)
    # preload the ACT function table (walrus inserts the ACT_TABLE_LOAD
    # before this first ACTIVATE) so it overlaps the input DMAs
    scratch = const_pool.tile([128, 1], F16)
    nc.vector.memset(scratch, 0.0)
    nc.scalar.activation(out=scratch, in_=scratch,
                         func=mybir.ActivationFunctionType.Derivative_Erf)

    # ---- weights + params.  The first PE block's weights (kc 0..5) go on
    # the fast sync (HWDGE) queue ahead of the xT loads; the rest stream on
    # the gpsimd (SWDGE) queue and stay ahead of PE block consumption ----
    w_tile = w_pool.tile([128, K_SPLINE, OUT_DIM], F16)
    nc.sync.dma_start(out=w_tile[:, 0:KB, :], in_=w_sp[:, 0:KB, :])
    params_sb = const_pool.tile([128, N_DT + N_DT + N_OSUB + G], F32)
    nc.gpsimd.dma_start(out=params_sb, in_=params)
    uscale_sb = params_sb[:, 0:N_DT]
    ushift_sb = params_sb[:, N_DT:2 * N_DT]
    bias_sb = params_sb[:, 2 * N_DT:2 * N_DT + N_OSUB]
    negc = params_sb[:, 2 * N_DT + N_OSUB:]
    for ws in range(KB, K_SPLINE, W_SLICE):
        we = min(ws + W_SLICE, K_SPLINE)
        nc.gpsimd.dma_start(out=w_tile[:, ws:we, :], in_=w_sp[:, ws:we, :])
    wb_tile = w_pool.tile([128, K_BASE, OUT_DIM], F16)
    nc.gpsimd.dma_start(out=wb_tile, in_=w_b)

    def emit_producers(ch):
        b0 = ch * b_chunk
        xts, relus, basis = [], [], []
        for dt in range(N_DT):
            xt = xt_pool.tile([128, b_chunk], F16, tag="xt")
            nc.sync.dma_start(
                out=xt,
                in_=x16t[dt * 128:(dt + 1) * 128, b0:b0 + b_chunk],
            )
            xts.append(xt)
            u = u_pool.tile([128, b_chunk], F32, tag="u")
            nc.vector.tensor_scalar(
                out=u, in0=xt,
                scalar1=uscale_sb[:, dt:dt + 1], scalar2=ushift_sb[:, dt:dt + 1],
                op0=mybir.AluOpType.mult, op1=mybir.AluOpType.add,
            )
            rl = relu_pool.tile([128, b_chunk], F16, tag="relu")
            nc.vector.tensor_scalar_max(out=rl, in0=xt, scalar1=0.0)
            relus.append(rl)
            for g in range(G):
                bt = basis_pool.tile([128, b_chunk], F16, tag="basis")
                # basis_g = sqrt(pi)/2 * d/dx erf(u - c_g); constant folded
                # into W_spline host-side.
                nc.scalar.activation(
                    out=bt, in_=u,
                    func=mybir.ActivationFunctionType.Derivative_Erf,
                    bias=negc[:, g:g + 1],
                )
                basis.append(bt)
        return xts, relus, basis

    def operands(kc, osub, xts, relus, basis):
        if kc < K_SPLINE:
            return w_tile[:, kc, osub * 128:(osub + 1) * 128], basis[kc]
        if kc < K_SPLINE + K_BASE:
            dt = kc - K_SPLINE
            return wb_tile[:, dt, osub * 128:(osub + 1) * 128], relus[dt]
        return ident, xts[osub]

    def emit_main_blocks(ch, psums, xts, relus, basis):
        for kb in range(0, K_LAST, KB):
            for osub in range(N_OSUB):
                for bh in range(n_bh):
                    for kc in range(kb, kb + KB):
                        lhsT, rhs = operands(kc, osub, xts, relus, basis)
                        nc.tensor.matmul(
                            psums[osub * n_bh + bh], lhsT=lhsT,
                            rhs=rhs[:, bh * 512:(bh + 1) * 512],
                            start=(kc == 0), stop=False)

    def emit_final_block(ch, psums, xts, relus, basis):
        b0 = ch * b_chunk
        for osub in range(N_OSUB):
            for bh in range(n_bh):
                ps = psums[osub * n_bh + bh]
                for kc in range(K_LAST, k_total):
                    lhsT, rhs = operands(kc, osub, xts, relus, basis)
                    nc.tensor.matmul(
                        ps, lhsT=lhsT, rhs=rhs[:, bh * 512:(bh + 1) * 512],
                        start=False, stop=(kc == k_total - 1))
                ot = out_pool.tile([128, 512], F32, tag="ot")
                nc.vector.tensor_scalar_add(
                    out=ot, in0=ps, scalar1=bias_sb[:, osub:osub + 1])
                nc.sync.dma_start(
                    out=out_t[osub * 128:(osub + 1) * 128,
                              b0 + bh * 512:b0 + (bh + 1) * 512],
                    in_=ot)

    # Emission order keeps each engine's in-order stream free of
    # cross-chunk serialization: chunk ch+1's DVE/ACT producer ops are
    # emitted BEFORE chunk ch's final block + epilogue (which wait on ch's
    # last matmuls).
    def alloc_psums(ch):
        return [psum_pool.tile([128, 512], F32, tag="ps", name=f"ps{ch}_{i}")
                for i in range(N_OSUB * n_bh)]

    psums0 = alloc_psums(0)
    # PE warm-up: ~32 dependency-free matmuls into psums0[0] release the HAM
    # clock throttle (~3.4us of sustained PE activity) before the first real
    # matmul arrives; the real kc==0 matmul has start=True, which resets the
    # bank, so the junk results never reach the output.
    for _ in range(32):
        nc.tensor.matmul(psums0[0][:, 0:128], lhsT=ident, rhs=ident,
                         start=True, stop=False)

    prod = emit_producers(0)
    psums = psums0
    for ch in range(n_chunks):
        emit_main_blocks(ch, psums, *prod)
        cur_prod, cur_psums = prod, psums
        if ch + 1 < n_chunks:
            prod = emit_producers(ch + 1)
            psums = alloc_psums(ch + 1)
        emit_final_block(ch, cur_psums, *cur_prod)


def build_program(b_shard=B_SHARD, b_chunk=B_CHUNK):
    nc = bacc.Bacc("TRN2", target_bir_lowering=False, debug=False,
                   num_devices=N_CORES)
    x16t = nc.dram_tensor("x16t", [IN_DIM, b_shard], F16,
                          kind="ExternalInput").ap()
    w_sp = nc.dram_tensor("w_sp", [128, K_SPLINE, OUT_DIM], F16,
                          kind="ExternalInput").ap()
    w_b = nc.dram_tensor("w_base", [128, K_BASE, OUT_DIM], F16,
                         kind="ExternalInput").ap()
    n_par = 2 * N_DT + N_OSUB + G
    params = nc.dram_tensor("params", [128, n_par], F32,
                            kind="ExternalInput").ap()
    out_t = nc.dram_tensor("out_t", [OUT_DIM, b_shard], F32,
                           kind="ExternalOutput").ap()
    with tile.TileContext(nc) as tc:
        _body(tc, x16t, w_sp, w_b, params, out_t, b_shard, b_chunk)
    nc.compile()
    return nc


def make_in_maps(x, gamma, beta, moving_mean, moving_var, W_spline, b_spline,
                 W_base, b_base, n_cores=N_CORES):
    """Host-side preprocessing + per-core input shards."""
    x = np.asarray(x, dtype=np.float32)
    gamma = np.asarray(gamma, dtype=np.float32)
    beta = np.asarray(beta, dtype=np.float32)
    moving_mean = np.asarray(moving_mean, dtype=np.float32)
    moving_var = np.asarray(moving_var, dtype=np.float32)
    W_spline = np.asarray(W_spline, dtype=np.float32)
    W_base = np.asarray(W_base, dtype=np.float32)
    b_spline = np.asarray(b_spline, dtype=np.float32)
    b_base = np.asarray(b_base, dtype=np.float32)

    scale = gamma / np.sqrt(moving_var + np.float32(BN_EPS))
    shift = beta - moving_mean * scale
    uscale = (scale / np.float32(DENOM)).astype(np.float32)
    ushift = (shift / np.float32(DENOM)).astype(np.float32)

    x16t = np.ascontiguousarray(x.T.astype(np.float16))  # [D, B]
    # K-order on chip is (dt, g, d_in): kc = dt*8+g covers d in
    # [dt*128, (dt+1)*128) at grid g.  W_spline rows are (d, g)-ordered.
    w_r = (W_spline.reshape(N_DT, 128, G, OUT_DIM)
           .transpose(0, 2, 1, 3)            # (dt, g, d_in, o)
           .reshape(K_SPLINE, 128, OUT_DIM)
           .transpose(1, 0, 2))              # (d_in, kc, o)
    w_sp = np.ascontiguousarray(w_r * np.float32(np.sqrt(np.pi) / 2.0)
                               ).astype(np.float16)
    w_b = np.ascontiguousarray(
        W_base.reshape(K_BASE, 128, OUT_DIM).transpose(1, 0, 2)
    ).astype(np.float16)
    bias_o = (b_spline + b_base).astype(np.float32)
    c = _grid_consts()
    params = np.empty((128, 2 * N_DT + N_OSUB + G), np.float32)
    params[:, 0:N_DT] = uscale.reshape(N_DT, 128).T
    params[:, N_DT:2 * N_DT] = ushift.reshape(N_DT, 128).T
    params[:, 2 * N_DT:2 * N_DT + N_OSUB] = bias_o.reshape(N_OSUB, 128).T
    params[:, 2 * N_DT + N_OSUB:] = -c[None, :]

    b_shard = x.shape[0] // n_cores
    return [
        {
            "x16t": np.ascontiguousarray(
                x16t[:, ci * b_shard:(ci + 1) * b_shard]),
            "w_sp": w_sp,
            "w_base": w_b,
            "params": params,
        }
        for ci in range(n_cores)
    ]


_PROGRAM = None


def kernel(x, gamma, beta, moving_mean, moving_var, W_spline, b_spline,
           W_base, b_base):
    global _PROGRAM
    if _PROGRAM is None:
        _PROGRAM = build_program()
    in_maps = make_in_maps(x, gamma, beta, moving_mean, moving_var,
                           W_spline, b_spline, W_base, b_base)
    res = run_bass_kernel_spmd(_PROGRAM, in_maps, core_ids=list(range(N_CORES)))
    out = np.concatenate(
        [np.ascontiguousarray(res.results[ci]["out_t"].T)
         for ci in range(N_CORES)], axis=0)
    return out.astype(np.float32)


# revision 11
# speedup vs baseline: 1.1797x; 1.0246x over previous
"""Trainium2 Bass kernel for EnhancedFastKANLayer.

Reference computation (B=16384, D=O=512, G=8 grids):
    x_norm = (x - mean) * rsqrt(var + eps) * gamma + beta          # BN inference
    basis[b,d,g] = exp(-((x_norm[b,d] - grid[g]) / denom)^2)       # RBF expansion
    out = basis.reshape(B, D*G) @ W_spline + b_spline
        + relu(x) @ W_base + b_base + x

Strategy:
  - Data parallel: batch 16384 sharded 8 ways (2048 rows/core); weights
    replicated. No collectives.
  - All on-chip compute happens in the transposed layout [feature, batch]:
    the output is produced as out_T [O, B_shard] and transposed back on the
    host. This makes BN/basis per-partition-scalar ops, lets the spline
    matmul consume basis tiles directly as the moving operand, and makes the
    residual (+x) a single extra identity matmul into the same PSUM
    accumulator.
  - RBF via ScalarE Derivative_Erf: d/dx erf(x) = 2/sqrt(pi)*exp(-x^2), so
    basis_g = sqrt(pi)/2 * Derivative_Erf(u - c_g) -- ONE ACT op per grid
    (bias supplies -c_g), with the sqrt(pi)/2 constant folded into W_spline
    on the host.  HW-probed: rel err 1.3e-5, saturates cleanly to 0 for
    |x| > 6, no NaN/Inf out to |x|=24.
  - x is pre-cast to fp16 AND pre-transposed to [D, B_shard] on the host:
    fp16 keeps the BN input error at 2^-11 relative, and host-side
    transposition means every device DMA is a plain contiguous copy (no
    xbar DMA-transpose -> no transpose<->copy mode serialization on the
    DMA rings).
  - W_spline is pre-reordered on the host to K-order (dt, g, d_in) matching
    the order basis tiles are produced on chip, cast to bf16.
  - Matmul: out_T[o_sub, b] accumulates 37 matmuls per PSUM tile:
    32 spline K-chunks + 4 relu(x)@W_base K-chunks + 1 identity*xT residual.
    All 8 PSUM banks accumulate one chunk concurrently, K-blocked (blocks of
    6 K-chunks swept across all banks) so the PE starts as soon as the first
    6 basis tiles exist instead of waiting for the whole chunk's basis.
"""

import numpy as np
import ml_dtypes
from contextlib import ExitStack

import concourse.bass as bass
import concourse.tile as tile
from concourse import bacc, mybir
from concourse._compat import with_exitstack
from concourse.bass_utils import run_bass_kernel_spmd
from concourse.masks import make_identity

N_CORES = 8
BATCH, IN_DIM, OUT_DIM, G = 16384, 512, 512, 8
B_SHARD = BATCH // N_CORES          # 2048
B_CHUNK = 1024                      # batch columns processed per chunk
GRID_MIN, GRID_MAX, BN_EPS = -2.0, 2.0, 1e-3
DENOM = (GRID_MAX - GRID_MIN) / G   # 0.5
N_DT = IN_DIM // 128                # 4 d-tiles
K_SPLINE = N_DT * G                 # 32 spline K-chunks
K_BASE = N_DT                       # 4 base K-chunks
N_OSUB = OUT_DIM // 128             # 4 output partition tiles

F32 = mybir.dt.float32
F16 = mybir.dt.float16
BF16 = mybir.dt.bfloat16


def _grid_consts():
    grid = np.linspace(GRID_MIN, GRID_MAX, G, dtype=np.float32)
    c = (grid / np.float32(DENOM)).astype(np.float32)        # grid in u-units
    return c


def _col(vec_ap, start, p=128):
    """View rows [start, start+p) of a 1-D DRAM tensor as a [p, 1] AP."""
    return bass.AP(
        tensor=vec_ap.tensor,
        offset=vec_ap.offset + start,
        ap=[[1, p], [0, 1]],
    )


@with_exitstack
def _body(ctx, tc, x16t, w_sp, w_b, params, out_t, b_shard, b_chunk):
    nc = tc.nc
    n_chunks = b_shard // b_chunk
    n_bh = b_chunk // 512            # 512-wide moving-operand slices
    k_total = K_SPLINE + K_BASE + 1  # + residual identity matmul
    KB = 6                           # K-chunks per PE block
    K_LAST = 30                      # kc >= K_LAST run per-bank, fused w/ epilogue
    W_SLICE = 8                      # spline weight K-chunks per DMA

    const_pool = ctx.enter_context(tc.tile_pool(name="const", bufs=1))
    w_pool = ctx.enter_context(tc.tile_pool(name="w", bufs=1))
    xt_pool = ctx.enter_context(tc.tile_pool(name="xt", bufs=2 * N_DT))
    u_pool = ctx.enter_context(tc.tile_pool(name="u", bufs=3))
    # spline basis tiles: 32 per chunk stay resident through the chunk's
    # matmul phase; extra slots let the next chunk's production run ahead.
    basis_pool = ctx.enter_context(tc.tile_pool(name="basis", bufs=K_SPLINE + 12))
    relu_pool = ctx.enter_context(tc.tile_pool(name="relu", bufs=2 * N_DT))
    psum_pool = ctx.enter_context(
        tc.tile_pool(name="psum", bufs=8, space="PSUM"))
    out_pool = ctx.enter_context(tc.tile_pool(name="outs", bufs=4))

    # ---- identity first: it feeds the PE warm-up matmuls ----
    ident = const_pool.tile([128, 128], F16)
    make_identity(nc, ident)
    # preload the ACT function table (walrus inserts the ACT_TABLE_LOAD
    # before this first ACTIVATE) so it overlaps the input DMAs
    scratch = const_pool.tile([128, 1], F16)
    nc.vector.memset(scratch, 0.0)
    nc.scalar.activation(out=scratch, in_=scratch,
                         func=mybir.ActivationFunctionType.Derivative_Erf)

    # ---- weights + params.  The first PE block's weights (kc 0..5) go on
    # the fast sync (HWDGE) queue ahead of the xT loads; the rest stream on
    # the gpsimd (SWDGE) queue and stay ahead of PE block consumption ----
    w_tile = w_pool.tile([128, K_SPLINE, OUT_DIM], F16)
    nc.sync.dma_start(out=w_tile[:, 0:KB, :], in_=w_sp[:, 0:KB, :])
    params_sb = const_pool.tile([128, N_DT + N_DT + N_OSUB + G], F32)
    nc.gpsimd.dma_start(out=params_sb, in_=params)
    uscale_sb = params_sb[:, 0:N_DT]
    ushift_sb = params_sb[:, N_DT:2 * N_DT]
    bias_sb = params_sb[:, 2 * N_DT:2 * N_DT + N_OSUB]
    negc = params_sb[:, 2 * N_DT + N_OSUB:]
    for ws in range(KB, K_SPLINE, W_SLICE):
        we = min(ws + W_SLICE, K_SPLINE)
        nc.gpsimd.dma_start(out=w_tile[:, ws:we, :], in_=w_sp[:, ws:we, :])
    wb_tile = w_pool.tile([128, K_BASE, OUT_DIM], F16)
    nc.gpsimd.dma_start(out=wb_tile, in_=w_b)

    def emit_producers(ch):
        b0 = ch * b_chunk
        xts, relus, basis = [], [], []
        for dt in range(N_DT):
            xt = xt_pool.tile([128, b_chunk], F16, tag="xt")
            nc.sync.dma_start(
                out=xt,
                in_=x16t[dt * 128:(dt + 1) * 128, b0:b0 + b_chunk],
            )
            xts.append(xt)
            u = u_pool.tile([128, b_chunk], F32, tag="u")
            nc.vector.tensor_scalar(
                out=u, in0=xt,
                scalar1=uscale_sb[:, dt:dt + 1], scalar2=ushift_sb[:, dt:dt + 1],
                op0=mybir.AluOpType.mult, op1=mybir.AluOpType.add,
            )
            rl = relu_pool.tile([128, b_chunk], F16, tag="relu")
            nc.vector.tensor_scalar_max(out=rl, in0=xt, scalar1=0.0)
            relus.append(rl)
            for g in range(G):
                bt = basis_pool.tile([128, b_chunk], F16, tag="basis")
                # basis_g = sqrt(pi)/2 * d/dx erf(u - c_g); constant folded
                # into W_spline host-side.
                nc.scalar.activation(
                    out=bt, in_=u,
                    func=mybir.ActivationFunctionType.Derivative_Erf,
                    bias=negc[:, g:g + 1],
                )
                basis.append(bt)
        return xts, relus, basis

    def operands(kc, osub, xts, relus, basis):
        if kc < K_SPLINE:
            return w_tile[:, kc, osub * 128:(osub + 1) * 128], basis[kc]
        if kc < K_SPLINE + K_BASE:
            dt = kc - K_SPLINE
            return wb_tile[:, dt, osub * 128:(osub + 1) * 128], relus[dt]
        return ident, xts[osub]

    def emit_main_blocks(ch, psums, xts, relus, basis):
        for kb in range(0, K_LAST, KB):
            for osub in range(N_OSUB):
                for bh in range(n_bh):
                    for kc in range(kb, kb + KB):
                        lhsT, rhs = operands(kc, osub, xts, relus, basis)
                        nc.tensor.matmul(
                            psums[osub * n_bh + bh], lhsT=lhsT,
                            rhs=rhs[:, bh * 512:(bh + 1) * 512],
                            start=(kc == 0), stop=False)

    def emit_final_block(ch, psums, xts, relus, basis):
        b0 = ch * b_chunk
        for osub in range(N_OSUB):
            for bh in range(n_bh):
                ps = psums[osub * n_bh + bh]
                for kc in range(K_LAST, k_total):
                    lhsT, rhs = operands(kc, osub, xts, relus, basis)
                    nc.tensor.matmul(
                        ps, lhsT=lhsT, rhs=rhs[:, bh * 512:(bh + 1) * 512],
                        start=False, stop=(kc == k_total - 1))
                ot = out_pool.tile([128, 512], F32, tag="ot")
                nc.vector.tensor_scalar_add(
                    out=ot, in0=ps, scalar1=bias_sb[:, osub:osub + 1])
                nc.sync.dma_start(
                    out=out_t[osub * 128:(osub + 1) * 128,
                              b0 + bh * 512:b0 + (bh + 1) * 512],
                    in_=ot)

    # Emission order keeps each engine's in-order stream free of
    # cross-chunk serialization: chunk ch+1's DVE/ACT producer ops are
    # emitted BEFORE chunk ch's final block + epilogue (which wait on ch's
    # last matmuls).
    def alloc_psums(ch):
        return [psum_pool.tile([128, 512], F32, tag="ps", name=f"ps{ch}_{i}")
                for i in range(N_OSUB * n_bh)]

    psums0 = alloc_psums(0)
    # PE warm-up: ~32 dependency-free matmuls into psums0[0] release the HAM
    # clock throttle (~3.4us of sustained PE activity) before the first real
    # matmul arrives; the real kc==0 matmul has start=True, which resets the
    # bank, so the junk results never reach the output.
    for _ in range(56):
        nc.tensor.matmul(psums0[0][:, 0:128], lhsT=ident, rhs=ident,
                         start=True, stop=False)

    prod = emit_producers(0)
    psums = psums0
    for ch in range(n_chunks):
        emit_main_blocks(ch, psums, *prod)
        cur_prod, cur_psums = prod, psums
        if ch + 1 < n_chunks:
            prod = emit_producers(ch + 1)
            psums = alloc_psums(ch + 1)
        emit_final_block(ch, cur_psums, *cur_prod)


def build_program(b_shard=B_SHARD, b_chunk=B_CHUNK):
    nc = bacc.Bacc("TRN2", target_bir_lowering=False, debug=False,
                   num_devices=N_CORES)
    x16t = nc.dram_tensor("x16t", [IN_DIM, b_shard], F16,
                          kind="ExternalInput").ap()
    w_sp = nc.dram_tensor("w_sp", [128, K_SPLINE, OUT_DIM], F16,
                          kind="ExternalInput").ap()
    w_b = nc.dram_tensor("w_base", [128, K_BASE, OUT_DIM], F16,
                         kind="ExternalInput").ap()
    n_par = 2 * N_DT + N_OSUB + G
    params = nc.dram_tensor("params", [128, n_par], F32,
                            kind="ExternalInput").ap()
    out_t = nc.dram_tensor("out_t", [OUT_DIM, b_shard], F32,
                           kind="ExternalOutput").ap()
    with tile.TileContext(nc) as tc:
        _body(tc, x16t, w_sp, w_b, params, out_t, b_shard, b_chunk)
    nc.compile()
    return nc


def make_in_maps(x, gamma, beta, moving_mean, moving_var, W_spline, b_spline,
                 W_base, b_base, n_cores=N_CORES):
    """Host-side preprocessing + per-core input shards."""
    x = np.asarray(x, dtype=np.float32)
    gamma = np.asarray(gamma, dtype=np.float32)
    beta = np.asarray(beta, dtype=np.float32)
    moving_mean = np.asarray(moving_mean, dtype=np.float32)
    moving_var = np.asarray(moving_var, dtype=np.float32)
    W_spline = np.asarray(W_spline, dtype=np.float32)
    W_base = np.asarray(W_base, dtype=np.float32)
    b_spline = np.asarray(b_spline, dtype=np.float32)
    b_base = np.asarray(b_base, dtype=np.float32)

    scale = gamma / np.sqrt(moving_var + np.float32(BN_EPS))
    shift = beta - moving_mean * scale
    uscale = (scale / np.float32(DENOM)).astype(np.float32)
    ushift = (shift / np.float32(DENOM)).astype(np.float32)

    x16t = np.ascontiguousarray(x.T.astype(np.float16))  # [D, B]
    # K-order on chip is (dt, g, d_in): kc = dt*8+g covers d in
    # [dt*128, (dt+1)*128) at grid g.  W_spline rows are (d, g)-ordered.
    w_r = (W_spline.reshape(N_DT, 128, G, OUT_DIM)
           .transpose(0, 2, 1, 3)            # (dt, g, d_in, o)
           .reshape(K_SPLINE, 128, OUT_DIM)
           .transpose(1, 0, 2))              # (d_in, kc, o)
    w_sp = np.ascontiguousarray(w_r * np.float32(np.sqrt(np.pi) / 2.0)
                               ).astype(np.float16)
    w_b = np.ascontiguousarray(
        W_base.reshape(K_BASE, 128, OUT_DIM).transpose(1, 0, 2)
    ).astype(np.float16)
    bias_o = (b_spline + b_base).astype(np.float32)
    c = _grid_consts()
    params = np.empty((128, 2 * N_DT + N_OSUB + G), np.float32)
    params[:, 0:N_DT] = uscale.reshape(N_DT, 128).T
    params[:, N_DT:2 * N_DT] = ushift.reshape(N_DT, 128).T
    params[:, 2 * N_DT:2 * N_DT + N_OSUB] = bias_o.reshape(N_OSUB, 128).T
    params[:, 2 * N_DT + N_OSUB:] = -c[None, :]

    b_shard = x.shape[0] // n_cores
    return [
        {
            "x16t": np.ascontiguousarray(
                x16t[:, ci * b_shard:(ci + 1) * b_shard]),
            "w_sp": w_sp,
            "w_base": w_b,
            "params": params,
        }
        for ci in range(n_cores)
    ]


_PROGRAM = None


def kernel(x, gamma, beta, moving_mean, moving_var, W_spline, b_spline,
           W_base, b_base):
    global _PROGRAM
    if _PROGRAM is None:
        _PROGRAM = build_program()
    in_maps = make_in_maps(x, gamma, beta, moving_mean, moving_var,
                           W_spline, b_spline, W_base, b_base)
    res = run_bass_kernel_spmd(_PROGRAM, in_maps, core_ids=list(range(N_CORES)))
    out = np.concatenate(
        [np.ascontiguousarray(res.results[ci]["out_t"].T)
         for ci in range(N_CORES)], axis=0)
    return out.astype(np.float32)
